# revision 35
# baseline (speedup 1.0000x reference)
"""Trainium2 Bass kernel for the CWICDense (conditional stripe matmul) module.

Problem (hardcoded shapes):
  x          [2, 512, 4096] f32    tokens T=1024, features I=4096
  W_kernel   [4096, 4096]   f32    viewed as [I, N=32 stripes, Q=128]
  thresholds [4096, 32]     f32
  mu         [4096]         f32    (structurally zero in this module)
  out_mu     [4096]         f32
  where      [2, 512]       bool   (unused by the reference computation)

  y[t, n*Q+q] = sum_i x_off[t,i] * (|x_off[t,i]| >= thresholds[i,n]) * W[i, n*Q+q]
                + out_mu[n*Q+q]

Sharding across 8 NeuronCores: 8-way tensor parallel over stripes (4 stripes
= 512 out cols per core); every core sees all 1024 tokens. The host passes
x pre-transposed (pure layout prep) so the device does no transposes at all,
and each core returns its y^T block which the host transposes back.

Per-core device algorithm (vs the 170 us f32r baseline -> ~108 us):
  - x^T k-tiles [128 features, 1024 tokens] f32, streamed/resident in SBUF.
  - per (stripe PAIR, k-tile): ONE custom DVE instruction over the
    zero-stride page view x[P, 2, T] computes
      z = select(x*x >= t2, x, 0)   with t2 = thresholds**2 (host-squared;
    equal to x * (|x| >= t) up to fp32 compare rounding at |x| == t —
    measure-zero). The op is registered with hand-authored perf-mode uop
    programs and perf_max=2, so the engine runs the 2X_2PORT program:
    2 elements/cycle with the stream SPLIT IN HALF across the two SBUF
    read ports (HW-verified semantics) — port 0 = page 0 (even stripe,
    compared against C0), port 1 = page 1 (odd stripe, against C1),
    results exiting via WR0/WR1 into the matching output halves. This
    takes the gate — the kernel's bottleneck — from 1 elem/cycle
    (2.4 us/pair-op) to 2 elems/cycle (1.34 us/pair-op). z is written
    as bf16.
  - PE matmul in bf16 (1 col/cycle vs f32r's 2, FWL weight loads; W is
    host-cast to bf16) accumulating y^T[n-block] fp32 in PSUM over 32
    k-tiles: acc += W[k,n].T @ z, two token-halves per stripe.
  - epilogue: ACT/DVE add out_mu (per-partition bias in y^T layout), DMA out.
  - numerics: bf16 z/W rounding gives rel err ~2.3e-3 vs the 2e-2 gate
    (fp32/f32r everywhere was 1.5e-4; fp16 x halves DMA but trips the
    broken-4x-slot hang and ~1e-2 gate flips — rejected).
  - DMA pacing note: the x stream (~16.8 MB/core) rides Q1's 16 engines
    at ~50% duty, throttled by xT pool recycling (bufs=18). Removing the
    throttle (bufs=32, or dual-queue issue) makes the DMA engines run
    hot and SLOWS the DVE gate ~15-20% via SBUF port contention — a net
    loss (measured 127-144 us). bufs=18 + single sync-queue issue is the
    empirical sweet spot.
"""

import sys

if "/opt/trn_rl_repo" not in sys.path:
    sys.path.insert(0, "/opt/trn_rl_repo")

import numpy as np

import concourse.bass as bass
import concourse.bass_isa as bass_isa
import concourse.mybir as mybir
import concourse.tile as tile
from concourse import bacc, bass_utils
from concourse import dve_ops as _dve_ops
from concourse.dve_spec import Spec, Src0, C0, C1, Zero, lower, select, sq, PageIdx
from concourse.dve_table_gen import dve_ver_for
from concourse.dve_uop import (
    ENABLE,
    AluInp,
    AluOp,
    DelayInp,
    DveOpSpec,
    InpSel,
    OutPath,
    OutSel,
    Trigger,
    UopConfig,
    UopDpConfig,
)

# ---- problem constants -------------------------------------------------
B, S, I, N, Q = 2, 512, 4096, 32, 128
T = B * S                 # 1024 tokens
OUT = N * Q               # 4096
NCORES = 8
NS = N // NCORES          # 4 stripes per core
OUT_C = NS * Q            # 512 out cols per core
KT = I // 128             # 32 contraction tiles
P = 128
HF = 2                    # token halves per matmul group (1024 -> 2 x 512)
TH = T // HF              # 512

_CACHE = {}


def _gate_sq_ref(in0, in1, s0, s1, imm2):
    # in0: [P, N] x; s0: [P, 1] squared threshold
    x = in0.astype(np.float32)
    t2 = np.asarray(s0, np.float32).reshape(-1, 1)
    return np.where(x * x >= t2, x, 0.0).astype(np.float32)


def _gate_2x_uop(two_port):
    """Steady uop computing z = select(x*x >= t2, x, 0) for TWO stream
    elements per cycle. Element 0 arrives on SRC_0; element 1 on SRC_1
    (2X_2PORT: the engine drives read port 1 over the same tensor) or on
    SRC_0_HI (2X_1PORT: packed 16-bit pair). Mirrors the structure of the
    stock TENSOR_SCALAR perf-mode programs (slots 17/18 of the firmware
    table)."""
    e1 = InpSel.SRC_1 if two_port else InpSel.SRC_0_HI
    u = UopConfig()
    u.enable_input(InpSel.SRC_0, 0)      # x0 -> blk0 ALU input
    u.enable_input(InpSel.CONST_0, 1)    # t2 -> d0
    u.enable_input(InpSel.ZERO, 2)       # 0  -> d1
    u.enable_input(InpSel.SRC_0, 3)      # x0 -> d2 (select's taken value)
    u.enable_input(e1, 4)                # x1 -> d3
    dp = u.datapath_config
    # blk0: x0^2
    dp[0].enable_alu(AluOp.MULTIPLY, AluInp.PREV_ALU_OUT, AluInp.PREV_ALU_OUT)
    dp[0].pass_through_delay(0, 1, 2, 3)
    # blk1: ge0 = x0^2 >= t2
    dp[1].enable_alu(AluOp.IS_GE, AluInp.PREV_ALU_OUT, AluInp.PREV_DELAY_0)
    dp[1].pass_through_delay(0, 1, 2, 3)
    # blk2: z0 = select(ge0, x0, 0)  (HW: src1 on true, src0 on false)
    dp[2].enable_alu(AluOp.SELECT, AluInp.PREV_DELAY_1, AluInp.PREV_DELAY_2)
    dp[2].pass_through_delay(0, 1, 3)
    # blk3: x1^2; capture z0 into d2 (x0 is dead there now)
    dp[3].enable_alu(AluOp.MULTIPLY, AluInp.PREV_DELAY_3, AluInp.PREV_DELAY_3)
    dp[3].enable_delay_from_src(DelayInp.PREV_ALU_OUT, 2)
    dp[3].pass_through_delay(0, 1, 3)
    # blk4: ge1 = x1^2 >= t2
    dp[4].enable_alu(AluOp.IS_GE, AluInp.PREV_ALU_OUT, AluInp.PREV_DELAY_0)
    dp[4].pass_through_delay(1, 2, 3)
    # blk5: z1 = select(ge1, x1, 0)
    dp[5].enable_alu(AluOp.SELECT, AluInp.PREV_DELAY_1, AluInp.PREV_DELAY_3)
    dp[5].pass_through_delay(2)
    # blk6/7: propagate z1 on the ALU chain, z0 rides d2
    dp[6].enable_alu(AluOp.BYPASS, AluInp.PREV_ALU_OUT)
    dp[6].pass_through_delay(2)
    dp[7].enable_alu(AluOp.BYPASS, AluInp.PREV_ALU_OUT)
    dp[7].pass_through_delay(2)
    u.enable_output(OutSel.DELAY_2, OutPath.WR0_LO)
    u.enable_output(
        OutSel.ALU_OUT, OutPath.WR1_LO if two_port else OutPath.WR0_HI
    )
    u.require_inp0 = ENABLE
    u.require_inp1 = ENABLE if two_port else 0
    u.trigger = (Trigger.SRC_TENSOR_DONE, Trigger.NONE, Trigger.NONE)
    return u


def _register_gate_sq_op():
    name = "CWIC_GATE_SQ_ANT"
    if name in _dve_ops._SUB_OPCODE_FOR_NAME:
        return next(op for op in _dve_ops.OPS if op.name == name)
    row = max(_dve_ops._SUB_OPCODE_FOR_NAME.values()) + 1
    assert row < 0x20
    _dve_ops._SUB_OPCODE_FOR_NAME[name] = row
    spec = Spec(
        body=select(sq(Src0) >= C0, Src0, Zero),
        reference=_gate_sq_ref,
    )
    uops_1x = lower(spec, ver="v3")
    assert len(uops_1x) == 1
    dspec = DveOpSpec(
        name=name,
        opcode=row,
        uops=uops_1x,
        uops_2x=[_gate_2x_uop(two_port=False)],
        uops_2x_2p=[_gate_2x_uop(two_port=True)],
        uops_4x=None,
        perf_max=2,
        rd1_en=False,
    )
    dspec.validate("v3")
    op = _dve_ops.DveOp(
        name, spec, subdim=False, uops_sha={"v3": dspec.sha("v3")}
    )
    _dve_ops.OPS.append(op)
    _dve_ops.CUSTOM_DVE_SPECS[name] = spec
    _dve_ops._COMPILE_CACHE[(name, "v3")] = dspec
    return op


def _gate2_sq_ref(in0, in1, s0, s1, imm2):
    # in0: [P, 2, N] x (page-broadcast); page 0 compares against s0
    # (t_even^2), page 1 against s1 (t_odd^2).
    x = in0.astype(np.float32)
    t2 = np.stack([np.asarray(s0, np.float32).reshape(-1, 1),
                   np.asarray(s1, np.float32).reshape(-1, 1)], axis=1)
    return np.where(x * x >= t2, x, 0.0).astype(np.float32)


def _gate2_1x_uop(state):
    """One state of the stripe-PAIR gate's REGULAR (1x) program: the
    sequential [P, 2, T] stream with the active threshold^2 held in
    blk0's CURR_ALU_OUT flop — seeded from C0, RELOADED from C1 at the
    page boundary (SUB_DIM_DONE -> step). Same FSM shape as lower()'s
    3-state PageIdx machine, but the step LOADS C1 rather than adding a
    delta, so the call-site scalars are the two raw squared thresholds
    for both the 1x and 2x programs."""
    u = UopConfig()
    u.enable_input(InpSel.SRC_0, 0)      # x -> blk0 ALU input (unused)
    u.enable_input(InpSel.CONST_0, 1)    # t_even^2 -> d0 (seed)
    u.enable_input(InpSel.CONST_1, 2)    # t_odd^2  -> d1 (step)
    u.enable_input(InpSel.ZERO, 3)       # 0 -> d2
    u.enable_input(InpSel.SRC_0, 4)      # x -> d3
    dp = u.datapath_config
    # blk0: the active t^2 flop (the only state-dependent block)
    if state in ("seed", "steady2"):
        dp[0].enable_alu(AluOp.BYPASS, AluInp.PREV_DELAY_0, AluInp.PREV_DELAY_0)
    elif state == "steady":
        dp[0].enable_alu(AluOp.BYPASS, AluInp.CURR_ALU_OUT, AluInp.CURR_ALU_OUT)
    else:  # step: entering page 1 -> load t_odd^2
        dp[0].enable_alu(AluOp.BYPASS, AluInp.PREV_DELAY_1, AluInp.PREV_DELAY_1)
    dp[0].pass_through_delay(0, 1, 2, 3)
    # blk1: x^2; capture t^2 into d4
    dp[1].enable_alu(AluOp.MULTIPLY, AluInp.PREV_DELAY_3, AluInp.PREV_DELAY_3)
    dp[1].enable_delay_from_src(DelayInp.PREV_ALU_OUT, 4)
    dp[1].pass_through_delay(2, 3)
    # blk2: ge = x^2 >= t^2
    dp[2].enable_alu(AluOp.IS_GE, AluInp.PREV_ALU_OUT, AluInp.PREV_DELAY_4)
    dp[2].pass_through_delay(2, 3)
    # blk3: z = select(ge, x, 0)
    dp[3].enable_alu(AluOp.SELECT, AluInp.PREV_DELAY_2, AluInp.PREV_DELAY_3)
    # blk4-7: propagate z
    for b in range(4, 8):
        dp[b].enable_alu(AluOp.BYPASS, AluInp.PREV_ALU_OUT)
    if state == "seed":
        u.require_inp0 = 0
        u.trigger = (Trigger.COUNT, Trigger.NONE, Trigger.NONE)
        u.next_uop = (1, 0, 0)
        u.repeat_count = 1
        return u
    u.enable_output(OutSel.ALU_OUT, OutPath.WR0_LO)
    u.require_inp0 = ENABLE
    if state == "steady2":
        # single-state chain (no page machinery; see _register_gate_sq2_op)
        u.trigger = (Trigger.SRC_TENSOR_DONE, Trigger.NONE, Trigger.NONE)
        u.next_uop = (0, 0, 0)
    elif state == "steady":
        u.trigger = (Trigger.SRC_TENSOR_DONE, Trigger.SUB_DIM_DONE, Trigger.NONE)
        u.next_uop = (0, 2, 0)
    else:  # step
        u.trigger = (Trigger.SRC_TENSOR_DONE, Trigger.SUB_DIM_DONE, Trigger.COUNT)
        u.next_uop = (0, 2, 1)
        u.repeat_count = 1
    return u


def _gate2_2x_uop(state, two_port):
    """One state of the stripe-PAIR gate's 2x program. HW-verified port
    semantics of 2X_2PORT: the stream is split IN HALF along the major
    dim — port 0 (SRC_0 lanes) streams the first half (= page 0, even
    stripe) while port 1 (SRC_1) streams the second half (= page 1, odd
    stripe), with WR0/WR1 writing the matching dst halves. So chain A
    compares against CONST_0 (t_even^2) and chain B against CONST_1
    (t_odd^2); no page-counter state is needed. All three states share
    this datapath — only the FSM flags mirror lower()'s 1x three-state
    chain (whose slot indices the mode tables must match)."""
    e1 = InpSel.SRC_1 if two_port else InpSel.SRC_0_HI
    u = UopConfig()
    u.enable_input(InpSel.SRC_0, 0)      # x0 -> blk0 ALU input
    u.enable_input(InpSel.CONST_0, 1)    # t_even^2 -> d0
    u.enable_input(InpSel.ZERO, 2)       # 0  -> d1
    u.enable_input(InpSel.SRC_0, 3)      # x0 -> d2
    u.enable_input(e1, 4)                # x1 -> d3
    u.enable_input(InpSel.CONST_1, 5)    # t_odd^2 -> d4
    dp = u.datapath_config
    # blk0: x0^2
    dp[0].enable_alu(AluOp.MULTIPLY, AluInp.PREV_ALU_OUT, AluInp.PREV_ALU_OUT)
    dp[0].pass_through_delay(0, 1, 2, 3, 4)
    # blk1: ge0 = x0^2 >= t_even^2
    dp[1].enable_alu(AluOp.IS_GE, AluInp.PREV_ALU_OUT, AluInp.PREV_DELAY_0)
    dp[1].pass_through_delay(1, 2, 3, 4)
    # blk2: z0 = select(ge0, x0, 0)
    dp[2].enable_alu(AluOp.SELECT, AluInp.PREV_DELAY_1, AluInp.PREV_DELAY_2)
    dp[2].pass_through_delay(1, 3, 4)
    # blk3: x1^2; capture z0 into d2 (x0 is dead)
    dp[3].enable_alu(AluOp.MULTIPLY, AluInp.PREV_DELAY_3, AluInp.PREV_DELAY_3)
    dp[3].enable_delay_from_src(DelayInp.PREV_ALU_OUT, 2)
    dp[3].pass_through_delay(1, 3, 4)
    # blk4: ge1 = x1^2 >= t_odd^2
    dp[4].enable_alu(AluOp.IS_GE, AluInp.PREV_ALU_OUT, AluInp.PREV_DELAY_4)
    dp[4].pass_through_delay(1, 2, 3)
    # blk5: z1 = select(ge1, x1, 0)
    dp[5].enable_alu(AluOp.SELECT, AluInp.PREV_DELAY_1, AluInp.PREV_DELAY_3)
    dp[5].pass_through_delay(2)
    # blk6/7: propagate z1; z0 rides d2
    dp[6].enable_alu(AluOp.BYPASS, AluInp.PREV_ALU_OUT)
    dp[6].pass_through_delay(2)
    dp[7].enable_alu(AluOp.BYPASS, AluInp.PREV_ALU_OUT)
    dp[7].pass_through_delay(2)
    if state == "seed":
        u.require_inp0 = 0
        u.require_inp1 = 0
        u.trigger = (Trigger.COUNT, Trigger.NONE, Trigger.NONE)
        u.next_uop = (1, 0, 0)
        u.repeat_count = 1
        return u
    u.enable_output(OutSel.DELAY_2, OutPath.WR0_LO)
    u.enable_output(
        OutSel.ALU_OUT, OutPath.WR1_LO if two_port else OutPath.WR0_HI
    )
    u.require_inp0 = ENABLE
    u.require_inp1 = ENABLE if two_port else 0
    u.trigger = (Trigger.SRC_TENSOR_DONE, Trigger.NONE, Trigger.NONE)
    u.next_uop = (0, 0, 0)
    return u


def _register_gate_sq2_op():
    name = "CWIC_GATE_SQ2_ANT"
    if name in _dve_ops._SUB_OPCODE_FOR_NAME:
        return next(op for op in _dve_ops.OPS if op.name == name)
    row = max(_dve_ops._SUB_OPCODE_FOR_NAME.values()) + 1
    assert row < 0x20
    _dve_ops._SUB_OPCODE_FOR_NAME[name] = row
    spec = Spec(
        body=select(sq(Src0) >= PageIdx(C0, C1), Src0, Zero),
        reference=_gate2_sq_ref,
    )
    # Single-uop chains: the 2X_2PORT program (the one that actually runs
    # for this fp32 stream) carries no cross-page state, so no seed/step
    # states are needed. The REGULAR slot is a best-effort single-state
    # program that would mis-gate page 1 — acceptable because the 2X_2PORT
    # triggers deterministically for this op's shape (fp32, SBUF, even
    # major dim, single-source); a fallback would be caught immediately by
    # the correctness check (stripe-level error), not silently.
    dspec = DveOpSpec(
        name=name,
        opcode=row,
        uops=[_gate2_1x_uop("steady2")],
        uops_2x=[_gate2_2x_uop("steady", False)],
        uops_2x_2p=[_gate2_2x_uop("steady", True)],
        uops_4x=None,
        perf_max=2,
        rd1_en=False,
    )
    dspec.validate("v3")
    op = _dve_ops.DveOp(
        name, spec, subdim=False, uops_sha={"v3": dspec.sha("v3")}
    )
    _dve_ops.OPS.append(op)
    _dve_ops.CUSTOM_DVE_SPECS[name] = spec
    _dve_ops._COMPILE_CACHE[(name, "v3")] = dspec
    return op


def _emit_gate2(vec, op, out, in0, s0, s1):
    """Emit the stripe-pair gate (subdim op: [P, 2, T] in/out, threshold
    pages advanced at SUB_DIM_DONE) with perf_max set."""
    b = vec.bass
    if op.name not in b.m.ant_custom_dve_ops:
        b.m.ant_custom_dve_ops = sorted({*b.m.ant_custom_dve_ops, op.name})
    shape = bass_isa.CustomDveShape.TTSS
    isa_opcode = b.isa.Opcode[
        f"NEURON_ISA_TPB_OPCODE_CUSTOM_DVE_ANT_{shape.slot()}"
    ].value
    ins = [
        vec.lower_ap(in0, for_isa=True, opt=False),
        vec.lower_ap(s0, for_isa=True),
        vec.lower_ap(s1, for_isa=True),
    ]
    return vec.add_instruction(
        bass_isa.InstCustomDveAnt(
            name=b.get_next_instruction_name(),
            op_name=op.name,
            rd1_en=False,
            subdim=0,
            imm2=0.0,
            shape=shape,
            row=_dve_ops.get_dve_sub_opcode(op.name),
            isa_opcode=isa_opcode,
            perf_max=2,
            ins=ins,
            outs=[vec.lower_ap(out, for_isa=True, opt=False)],
        )
    )


def _emit_gate(vec, op, out, in0, s0):
    """Emit one CWIC_GATE_SQ_ANT instruction with perf_max set so the
    engine may pick the 2x uop programs. Mirrors bass.Vector._custom_dve
    for the single-source TTSS case (that helper hardcodes perf_max=0)."""
    b = vec.bass
    if op.name not in b.m.ant_custom_dve_ops:
        b.m.ant_custom_dve_ops = sorted({*b.m.ant_custom_dve_ops, op.name})
    shape = bass_isa.CustomDveShape.TTSS
    isa_opcode = b.isa.Opcode[
        f"NEURON_ISA_TPB_OPCODE_CUSTOM_DVE_ANT_{shape.slot()}"
    ].value
    ins = [
        vec.lower_ap(in0, for_isa=True),
        vec.lower_ap(s0, for_isa=True),
        mybir.ImmediateValue(dtype=mybir.dt.float32, value=0.0),
    ]
    return vec.add_instruction(
        bass_isa.InstCustomDveAnt(
            name=b.get_next_instruction_name(),
            op_name=op.name,
            rd1_en=False,
            subdim=0,
            imm2=0.0,
            shape=shape,
            row=_dve_ops.get_dve_sub_opcode(op.name),
            isa_opcode=isa_opcode,
            perf_max=2,
            ins=ins,
            outs=[vec.lower_ap(out, for_isa=True)],
        )
    )


def _build():
    f32 = mybir.dt.float32
    bf16 = mybir.dt.bfloat16
    gate2_op = _register_gate_sq2_op()
    nc = bacc.Bacc("TRN2", target_bir_lowering=False, debug=False)

    xT_d = nc.dram_tensor("xT", [I, T], f32, kind="ExternalInput").ap()
    w_d = nc.dram_tensor("w", [I, OUT_C], bf16, kind="ExternalInput").ap()
    # thr holds the SQUARED threshold per stripe column
    thr_d = nc.dram_tensor("thr", [I, NS], f32, kind="ExternalInput").ap()
    mu_d = nc.dram_tensor("mu", [P, NS], f32, kind="ExternalInput").ap()
    yT_d = nc.dram_tensor("yT", [OUT_C, T], f32, kind="ExternalOutput").ap()

    # w_v[p, k, c] = w[k*128+p, c]
    w_v = w_d.rearrange("(k p) c -> p k c", p=P)

    with tile.TileContext(nc) as tc:
        with (
            tc.tile_pool(name="const", bufs=1) as constp,
            tc.tile_pool(name="xT", bufs=18) as xTp,
            tc.tile_pool(name="thr", bufs=KT) as thrp,
            tc.tile_pool(name="w", bufs=12) as wp,
            tc.tile_pool(name="z2", bufs=5) as z2p,
            tc.tile_pool(name="yT", bufs=8) as yTp,
            tc.tile_pool(name="acc", bufs=2 * NS, space="PSUM") as accp,
        ):
            # interleave per-k threshold + x loads so z(n=0, k=0) can start
            # as soon as the first pair lands (per-tile dep granularity)
            xT = []
            thrT = []

            def load_pair(k):
                tk = thrp.tile([P, NS], f32, tag="thr", name=f"thr{k}")
                nc.sync.dma_start(tk[:], thr_d[k * P:(k + 1) * P, :])
                xk = xTp.tile([P, T], f32, tag="xT", name=f"xk{k}")
                nc.sync.dma_start(xk[:], xT_d[k * P:(k + 1) * P, :])
                xT.append(xk)
                thrT.append(tk)

            for k in range(2):
                load_pair(k)

            # HAM warm-up: a short burst of throwaway matmuls keyed on a DMA
            # that lands early in the x stream, so the PE clock is at 2.4 GHz
            # as real matmuls arrive (a cold PE runs at 1.2 GHz). The burst
            # writes into an accumulator bank that the real k=0 matmul
            # (start=True) resets afterwards.
            warmsrc = constp.tile([P, TH], bf16, tag="warmsrc")
            nc.scalar.dma_start(
                warmsrc[:], xT_d[P:2 * P, 0:TH // 2].bitcast(bf16)
            )

            for k in range(2, KT):
                load_pair(k)

            # all W chunks issued upfront on the scalar engine's DMA queue;
            # transfers stream in consumption (r-major) order.
            # wc[p, kk*Q+q] = w[(r*KC+kk)*128+p, n*Q+q]
            KC = 8  # k-tiles per W chunk DMA
            wcs = {}
            for r in range(KT // KC):
                for n in range(NS):
                    wc = wp.tile([P, KC * Q], bf16, tag="w", name=f"wc{n}_{r}")
                    nc.scalar.dma_start(
                        wc[:].rearrange("p (k q) -> p k q", q=Q),
                        w_v[:, r * KC:(r + 1) * KC, n * Q:(n + 1) * Q],
                    )
                    wcs[(n, r)] = wc

            # out_mu is only needed at the epilogue — keep it off the
            # critical sync-queue head
            mu_sb = constp.tile([P, NS], f32, tag="mu")
            nc.scalar.dma_start(mu_sb[:], mu_d)

            accs = [
                accp.tile([P, TH], f32, tag="acc", name=f"acc{n}_{h}")
                for n in range(NS) for h in range(HF)
            ]
            for _ in range(6):
                nc.tensor.matmul(
                    accs[-1][:],
                    warmsrc[:, 0:P],
                    warmsrc[:],
                    start=True,
                    stop=True,
                )

            # k-outer: every x tile is consumed immediately by all 4 stripes
            # (DVE stripe-pair gates; GpSimd STT gates for the GP_K k-tiles),
            # so the x stream spreads over the whole kernel instead of piling
            # into the first stripe's window.
            def mm(n, k, z_ap):
                for h in range(HF):
                    nc.tensor.matmul(
                        accs[n * HF + h][:],
                        wcs[(n, k // KC)][:, (k % KC) * Q:(k % KC + 1) * Q],
                        z_ap[:, h * TH:(h + 1) * TH],
                        start=(k == 0),
                        stop=(k == KT - 1),
                    )

            def gate_pair(k, pair, h0, ht):
                """One pair-gate op over tokens [h0*TH, (h0+ht)*TH); feeds
                the matching token-half matmuls of both stripes."""
                tw = ht * TH
                xa = xT[k][:, h0 * TH:h0 * TH + tw]
                x_pg = bass.AP(xa.tensor, xa.offset,
                               [list(xa.ap[0]), [0, 2], list(xa.ap[1])])
                zt2 = z2p.tile([P, 2 * tw], bf16, tag="z2")
                _emit_gate2(
                    nc.vector, gate2_op,
                    out=zt2[:].rearrange("p (s t) -> p s t", s=2),
                    in0=x_pg,
                    s0=thrT[k][:, 2 * pair:2 * pair + 1],
                    s1=thrT[k][:, 2 * pair + 1:2 * pair + 2],
                )
                for s in range(2):
                    n = 2 * pair + s
                    for h in range(h0, h0 + ht):
                        nc.tensor.matmul(
                            accs[n * HF + h][:],
                            wcs[(n, k // KC)][:, (k % KC) * Q:(k % KC + 1) * Q],
                            zt2[:, s * tw + (h - h0) * TH:
                                   s * tw + (h - h0 + 1) * TH],
                            start=(k == 0),
                            stop=(k == KT - 1),
                        )

            for k in range(KT):
                for pair in range(NS // 2):
                    gate_pair(k, pair, 0, HF)
            # epilogue: + out_mu (per-partition in y^T layout), DMA out.
            # Bias-adds alternate between ACT and GpSimd, keeping the tail
            # off the (bottleneck) Vector queue entirely.
            for n in range(NS):
                for h in range(HF):
                    yt = yTp.tile([P, TH], f32, tag="yT")
                    if (n * HF + h) % 2 == 0:
                        nc.scalar.activation(
                            yt[:], accs[n * HF + h][:],
                            mybir.ActivationFunctionType.Identity,
                            bias=mu_sb[:, n:n + 1],
                        )
                    else:
                        nc.vector.tensor_scalar(
                            yt[:], accs[n * HF + h][:], mu_sb[:, n:n + 1],
                            None, op0=mybir.AluOpType.add,
                        )
                    nc.sync.dma_start(
                        yT_d[n * P:(n + 1) * P, h * TH:(h + 1) * TH], yt[:]
                    )
    nc.compile()
    return nc


def _get_nc():
    if "nc" not in _CACHE:
        _CACHE["nc"] = _build()
    return _CACHE["nc"]


def _make_in_maps(x, W_kernel, thresholds, mu, out_mu):
    xf = np.asarray(x, dtype=np.float32).reshape(T, I)
    xf = xf - np.asarray(mu, dtype=np.float32)[None, :]
    xT = np.ascontiguousarray(xf.T)
    import ml_dtypes
    W = np.asarray(W_kernel, np.float32).astype(ml_dtypes.bfloat16)
    thr = np.asarray(thresholds, np.float32)
    thr2 = (thr * thr).astype(np.float32)
    omu = np.asarray(out_mu, np.float32)
    in_maps = []
    for g in range(NCORES):
        in_maps.append({
            "xT": xT,
            "w": np.ascontiguousarray(W[:, g * OUT_C:(g + 1) * OUT_C]),
            "thr": np.ascontiguousarray(thr2[:, g * NS:(g + 1) * NS]),
            "mu": np.ascontiguousarray(
                omu[g * OUT_C:(g + 1) * OUT_C].reshape(NS, P).T
            ),
        })
    return in_maps


def _assemble(results):
    yT = np.concatenate([results[g]["yT"] for g in range(NCORES)], axis=0)
    return np.ascontiguousarray(yT.T).reshape(B, S, OUT)


def run(inputs, **spmd_kwargs):
    """Run on hardware; returns (y, BassKernelResults)."""
    nc = _get_nc()
    in_maps = _make_in_maps(
        inputs["x"], inputs["W_kernel"], inputs["thresholds"],
        inputs["mu"], inputs["out_mu"],
    )
    res = bass_utils.run_bass_kernel_spmd(
        nc, in_maps, core_ids=list(range(NCORES)), **spmd_kwargs
    )
    return _assemble(res.results), res


def kernel(x, W_kernel, thresholds, mu, out_mu, where):
    y, _ = run({
        "x": x, "W_kernel": W_kernel, "thresholds": thresholds,
        "mu": mu, "out_mu": out_mu, "where": where,
    })
    return y
